# revision 5
# baseline (speedup 1.0000x reference)
"""Trainium2 Bass kernel for nn_CP_TransformerDecoder_Action.

Strategy
--------
Host side (numpy, not timed):
  * CP adapters + LN affine fold exactly into dense per-layer weights.
  * TP=8 tensor-parallel over heads / hidden across all 8 cores, with
    Megatron-style sequence parallelism: the residual stream is token-sharded
    (each core owns 128 tokens of each batch element, fp32, feature-major).
  * Weights are int8 with per-input-channel scales (dequantized to bf16 on
    device), x in/out are fp16 token shards -> total H2D+D2H ~57 MB instead
    of ~270 MB.

Device (one SPMD program, 8 cores):
  per layer, two independent per-batch streams interleaved so collectives
  overlap the other batch's compute:
    LN1(local 128 tokens) -> AllGather(bf16) -> qkv -> causal attention
    (exp trick with ones-column denominator) -> proj partial ->
    ReduceScatter(add) -> residual -> LN2 -> AllGather -> fc1+gelu ->
    fc2 partial -> ReduceScatter -> residual.
  Causal masks are generated on device (gpsimd affine_select) at prologue.
"""

import numpy as np
import ml_dtypes

L, B, N, C, H, D, R = 4, 2, 1024, 1024, 16, 64, 64
HID = 4 * C
NCORES = 8
TP = 8
KT = C // 128               # 8 feature tiles
HL = H // TP                # 2 heads per core
CL = HL * D                 # 128 local attention features
HIDL = HID // TP            # 512 local hidden
TB = N // TP                # 128 tokens per core per batch
CHUNK = 512                 # query-chunk for attention / matmul moving dim
NCHUNK = N // CHUNK         # 2 chunks per batch
VS = D + 4                  # v storage stride per head (64 data + 1 ones + pad)
RG = [[0, 1, 2, 3, 4, 5, 6, 7]]
SCQK, SCV, SCPR, SCF1, SCF2 = 0, 8, 16, 17, 25   # scale column offsets
SC_COLS = 29

BF16 = ml_dtypes.bfloat16
F16 = np.float16


def _fold_weights(inp):
    """Fold LN affine + CP adapters into dense per-layer weights (fp32 exact)."""
    f32 = np.float32
    u_w = np.asarray(inp['u_w'], f32)       # [R, C]
    v_w = np.asarray(inp['v_w'], f32)       # [C, R]
    cp_c = np.asarray(inp['cp_c'], f32)     # [R, R, R]
    out = []
    for l in range(L):
        g1 = np.asarray(inp['ln1_g'][l], f32); b1 = np.asarray(inp['ln1_b'][l], f32)
        g2 = np.asarray(inp['ln2_g'][l], f32); b2 = np.asarray(inp['ln2_b'][l], f32)
        qkv_w = np.asarray(inp['qkv_w'][l], f32)
        proj_w = np.asarray(inp['proj_w'][l], f32)
        fc1_w = np.asarray(inp['fc1_w'][l], f32)
        fc2_w = np.asarray(inp['fc2_w'][l], f32)
        CPa = np.einsum('abr,rf->abf', cp_c, np.asarray(inp['cp_att'][l], f32))
        CPm = np.einsum('abr,rf->abf', cp_c, np.asarray(inp['mlp_cp'][l], f32))

        Pcat = np.concatenate([CPa[:, :, i] @ v_w.T for i in range(3)], axis=1)   # [R,3C]
        Wqkv_t = (qkv_w * g1[None, :]).T + (u_w * g1[None, :]).T @ Pcat           # [C,3C]
        bqkv = b1 @ qkv_w.T + (b1 @ u_w.T) @ Pcat                                  # [3C]

        Wproj_t = proj_w.T + u_w.T @ (CPa[:, :, 3] @ v_w.T)                        # [C,C]
        bproj = np.asarray(inp['proj_b'][l], f32)

        fc1_cp = CPm[:, :, :4].reshape(R, 4 * R)
        T = np.concatenate([fc1_cp[:, j*R:(j+1)*R] @ v_w.T for j in range(4)], axis=1)
        Wfc1_t = (fc1_w * g2[None, :]).T + (u_w * g2[None, :]).T @ T               # [C,HID]
        bfc1 = np.asarray(inp['fc1_b'][l], f32) + b2 @ fc1_w.T + (b2 @ u_w.T) @ T

        fc2_cp = CPm[:, :, 4:].reshape(R, 4 * R)
        Z = np.concatenate([u_w.T @ fc2_cp[:, j*R:(j+1)*R].T @ v_w.T for j in range(4)], axis=0)
        Wfc2_t = fc2_w.T + Z                                                       # [HID,C]
        bfc2 = np.asarray(inp['fc2_b'][l], f32)
        out.append(dict(Wqkv_t=Wqkv_t, bqkv=bqkv, Wproj_t=Wproj_t, bproj=bproj,
                        Wfc1_t=Wfc1_t, bfc1=bfc1, Wfc2_t=Wfc2_t, bfc2=bfc2))
    return out


def _q8rows(w):
    """Per-row symmetric int8 quantization of w [rows, cols].

    Returns (q int8, sinv fp32[rows]) with w ~= q * sinv[:, None].
    """
    mx = np.abs(w).max(axis=1)
    s = np.where(mx > 0, 127.0 / np.maximum(mx, 1e-30), 0.0)
    q = np.clip(np.rint(w * s[:, None]), -127, 127).astype(np.int8)
    sinv = np.where(mx > 0, mx / 127.0, 0.0).astype(np.float32)
    return q, sinv


def build_program(bias_on, gelu_mode="exact", collective_mode="on", loop_mult=1):
    """Build the SPMD Bass/Tile program (TP=8, sequence-parallel)."""
    from contextlib import ExitStack
    import concourse.mybir as mybir
    import concourse.tile as tile
    from concourse import bacc

    dt = mybir.dt
    AF = mybir.ActivationFunctionType
    nc = bacc.Bacc(num_devices=NCORES)

    x_p = nc.declare_dram_parameter("x16", [KT, 128, 2 * TB], dt.float16, isOutput=False)
    wqk_p = nc.declare_dram_parameter("wqk", [L, KT, 128, 2 * CL], dt.int8, isOutput=False)
    wv_p = nc.declare_dram_parameter("wv", [L, KT, 128, CL], dt.int8, isOutput=False)
    wpr_p = nc.declare_dram_parameter("wpr", [L, 128, C], dt.int8, isOutput=False)
    wf1_p = nc.declare_dram_parameter("wf1", [L, KT, 128, HIDL], dt.int8, isOutput=False)
    wf2_p = nc.declare_dram_parameter("wf2", [L, HIDL // 128, 128, C], dt.int8, isOutput=False)
    sc_p = nc.declare_dram_parameter("sc", [L, 128, SC_COLS], dt.float32, isOutput=False)
    bias_p = {}
    for nm, shp in (("bqk", [L, 128, 2]), ("bv", [L, 128, CL]),
                    ("bpr", [L, 128, KT]), ("bf1", [L, 128, HIDL // 128]),
                    ("bf2", [L, 128, KT])):
        if bias_on[nm]:
            bias_p[nm] = nc.declare_dram_parameter(nm, shp, dt.float32, isOutput=False)
    out_p = nc.declare_dram_parameter("out", [KT, 128, 2 * TB], dt.float16, isOutput=True)

    with tile.TileContext(nc) as tc, ExitStack() as ctx:
        consts = ctx.enter_context(tc.tile_pool(name="consts", bufs=1))
        w8pool = ctx.enter_context(tc.tile_pool(name="w8pool", bufs=2))
        wpool = ctx.enter_context(tc.tile_pool(name="wpool", bufs=2))
        xpool = ctx.enter_context(tc.tile_pool(name="xpool", bufs=1))
        hpool = ctx.enter_context(tc.tile_pool(name="hpool", bufs=1))    # xh AG'd
        bpool = ctx.enter_context(tc.tile_pool(name="bpool", bufs=2))    # xb local LN out
        apool = ctx.enter_context(tc.tile_pool(name="apool", bufs=1))    # qT/kT/v/ot
        gpool = ctx.enter_context(tc.tile_pool(name="gpool", bufs=1))    # gelu acts
        espool = ctx.enter_context(tc.tile_pool(name="espool", bufs=3))
        stpool = ctx.enter_context(tc.tile_pool(name="stpool", bufs=4))  # RS staging
        upool = ctx.enter_context(tc.tile_pool(name="upool", bufs=4))    # RS results
        spool = ctx.enter_context(tc.tile_pool(name="spool", bufs=2))    # small stats
        sqpool = ctx.enter_context(tc.tile_pool(name="sqpool", bufs=3))
        ps_mm = ctx.enter_context(tc.tile_pool(name="ps_mm", bufs=3, space="PSUM"))
        ps_ot = ctx.enter_context(tc.tile_pool(name="ps_ot", bufs=2, space="PSUM"))
        ps_bc = ctx.enter_context(tc.tile_pool(name="ps_bc", bufs=2, space="PSUM"))
        ps_st = ctx.enter_context(tc.tile_pool(name="ps_st", bufs=1, space="PSUM"))
        dram = ctx.enter_context(tc.tile_pool(name="dram", bufs=2, space="DRAM"))

        # ---- constants
        ones_col = consts.tile([128, 1], dt.bfloat16)
        nc.vector.memset(ones_col, 1.0)
        ones_row = consts.tile([1, 128], dt.bfloat16)
        nc.vector.memset(ones_row, 1.0)
        eps_t = consts.tile([1, 1], dt.float32)
        nc.vector.memset(eps_t, 1e-5)
        # causal mask multipliers, generated on device: mask_t[p][kk, qq] =
        # (p*128 + kk) <= qq
        mask_t = []
        for p in range(CHUNK // 128):
            mt_ = consts.tile([128, CHUNK], dt.bfloat16, name=f"mask{p}")
            nc.vector.memset(mt_, 1.0)
            nc.gpsimd.affine_select(out=mt_, in_=mt_, pattern=[[1, CHUNK]],
                                    base=-(p * 128), channel_multiplier=-1,
                                    compare_op=mybir.AluOpType.is_ge, fill=0.0)
            mask_t.append(mt_)

        # ---- residual stream: my token blocks (b0|b1), feature-major fp32
        xt = []
        for k in range(KT):
            x16 = consts.tile([128, 2 * TB], dt.float16, name=f"xin{k}")
            nc.sync.dma_start(out=x16, in_=x_p[k])
            t = xpool.tile([128, 2 * TB], dt.float32, name=f"x{k}")
            nc.vector.tensor_copy(out=t, in_=x16)
            xt.append(t)

        def load_weights(l):
            sc = wpool.tile([128, SC_COLS], dt.float32, name="sct", tag="sct")
            nc.sync.dma_start(out=sc, in_=sc_p[l])
            wqk, wv, wf1, wf2 = [], [], [], []
            for k in range(KT):
                t8 = w8pool.tile([128, 2 * CL], dt.int8, name=f"wqk8_{k}", tag=f"wqk8_{k}")
                nc.sync.dma_start(out=t8, in_=wqk_p[l, k])
                t = wpool.tile([128, 2 * CL], dt.bfloat16, name=f"wqk{k}", tag=f"wqk{k}")
                nc.vector.tensor_scalar_mul(out=t, in0=t8, scalar1=sc[:, SCQK+k:SCQK+k+1])
                wqk.append(t)
            for k in range(KT):
                t8 = w8pool.tile([128, CL], dt.int8, name=f"wv8_{k}", tag=f"wv8_{k}")
                nc.sync.dma_start(out=t8, in_=wv_p[l, k])
                t = wpool.tile([128, CL], dt.bfloat16, name=f"wv{k}", tag=f"wv{k}")
                nc.vector.tensor_scalar_mul(out=t, in0=t8, scalar1=sc[:, SCV+k:SCV+k+1])
                wv.append(t)
            t8 = w8pool.tile([128, C], dt.int8, name="wpr8", tag="wpr8")
            nc.sync.dma_start(out=t8, in_=wpr_p[l])
            wpr = wpool.tile([128, C], dt.bfloat16, name="wpr", tag="wpr")
            nc.vector.tensor_scalar_mul(out=wpr, in0=t8, scalar1=sc[:, SCPR:SCPR+1])
            for k in range(KT):
                t8 = w8pool.tile([128, HIDL], dt.int8, name=f"wf18_{k}", tag=f"wf18_{k}")
                nc.sync.dma_start(out=t8, in_=wf1_p[l, k])
                t = wpool.tile([128, HIDL], dt.bfloat16, name=f"wf1{k}", tag=f"wf1{k}")
                nc.scalar.activation(out=t, in_=t8, func=AF.Copy,
                                     scale=sc[:, SCF1+k:SCF1+k+1])
                wf1.append(t)
            for j in range(HIDL // 128):
                t8 = w8pool.tile([128, C], dt.int8, name=f"wf28_{j}", tag=f"wf28_{j}")
                nc.sync.dma_start(out=t8, in_=wf2_p[l, j])
                t = wpool.tile([128, C], dt.bfloat16, name=f"wf2{j}", tag=f"wf2{j}")
                nc.scalar.activation(out=t, in_=t8, func=AF.Copy,
                                     scale=sc[:, SCF2+j:SCF2+j+1])
                wf2.append(t)
            bias_t = {}
            for nm in bias_p:
                t = wpool.tile(list(bias_p[nm].shape[1:]), dt.float32,
                               name=f"{nm}t", tag=f"{nm}t")
                nc.sync.dma_start(out=t, in_=bias_p[nm][l])
                bias_t[nm] = t
            return dict(wqk=wqk, wv=wv, wpr=wpr, wf1=wf1, wf2=wf2, bias=bias_t)

        def layer_norm(b, tag):
            """LN over my TB tokens of batch b; returns 8 bf16 [128, TB] tiles."""
            cs = slice(b * TB, (b + 1) * TB)
            xb = []
            for k in range(KT):
                t = bpool.tile([128, TB], dt.bfloat16, name=f"xb{k}", tag=f"xb{b}_{k}")
                nc.vector.tensor_copy(out=t, in_=xt[k][:, cs])
                xb.append(t)
            stat = ps_st.tile([64, TB], dt.float32, tag="stat")
            for k in range(KT):
                nc.tensor.matmul(stat[0:1, :], ones_col, xb[k],
                                 start=(k == 0), stop=(k == KT - 1))
            for k in range(KT):
                sq = sqpool.tile([128, TB], dt.bfloat16, name="sq", tag="sq")
                nc.vector.tensor_mul(out=sq, in0=xb[k], in1=xb[k])
                nc.tensor.matmul(stat[32:33, :], ones_col, sq,
                                 start=(k == 0), stop=(k == KT - 1))
            s1 = spool.tile([1, TB], dt.float32, tag="s1", bufs=2)   # negmean
            nc.scalar.activation(out=s1, in_=stat[0:1, :], func=AF.Copy,
                                 scale=-1.0 / C)
            s2 = spool.tile([1, TB], dt.float32, tag="s2", bufs=2)   # mean^2
            nc.vector.tensor_mul(out=s2, in0=s1, in1=s1)
            s3 = spool.tile([1, TB], dt.float32, tag="s3", bufs=2)   # msq
            nc.scalar.activation(out=s3, in_=stat[32:33, :], func=AF.Copy,
                                 scale=1.0 / C)
            nc.vector.tensor_sub(out=s3, in0=s3, in1=s2)             # var
            nc.scalar.activation(out=s2, in_=s3, func=AF.Sqrt, bias=eps_t[:, 0:1])
            nc.vector.reciprocal(out=s3, in_=s2)                     # rstd
            nc.vector.tensor_mul(out=s1, in0=s1, in1=s3)             # -mean*rstd
            a_bf = spool.tile([1, TB], dt.bfloat16, tag="a_bf", bufs=2)
            nc.scalar.activation(out=a_bf, in_=s3, func=AF.Copy)
            b_bf = spool.tile([1, TB], dt.bfloat16, tag="b_bf", bufs=2)
            nc.scalar.activation(out=b_bf, in_=s1, func=AF.Copy)
            ps_a = ps_bc.tile([128, TB], dt.float32, tag="bc")
            nc.tensor.matmul(ps_a, ones_row, a_bf, start=True, stop=True)
            a_bc = spool.tile([128, TB], dt.bfloat16, tag="a_bc", bufs=2)
            nc.scalar.activation(out=a_bc, in_=ps_a, func=AF.Copy)
            ps_b = ps_bc.tile([128, TB], dt.float32, tag="bc")
            nc.tensor.matmul(ps_b, ones_row, b_bf, start=True, stop=True)
            b_bc = spool.tile([128, TB], dt.bfloat16, tag="b_bc", bufs=2)
            nc.scalar.activation(out=b_bc, in_=ps_b, func=AF.Copy)
            for k in range(KT):
                nc.vector.tensor_mul(out=xb[k], in0=xb[k], in1=a_bc)
                nc.vector.tensor_add(out=xb[k], in0=xb[k], in1=b_bc)
            return xb

        def all_gather(b, xb):
            """AllGather my LN'd block -> full [C, N] bf16 xh tiles for batch b."""
            agin = dram.tile([KT, 128, TB], dt.bfloat16, name="agin", tag=f"agin{b}")
            for k in range(KT):
                nc.sync.dma_start(out=agin[k], in_=xb[k])
            agout = dram.tile([NCORES, KT, 128, TB], dt.bfloat16, name="agout",
                              tag=f"agout{b}", addr_space="Shared")
            if collective_mode == "on":
                nc.gpsimd.collective_compute(
                    "AllGather", mybir.AluOpType.bypass, replica_groups=RG,
                    ins=[agin.opt()], outs=[agout.opt()])
            else:
                for r in range(NCORES):
                    nc.gpsimd.dma_start(out=agout[r], in_=agin.opt())
            ago = agout.rearrange("r k p c -> k p r c")
            xh = []
            for k in range(KT):
                t = hpool.tile([128, N], dt.bfloat16, name=f"xh{k}", tag=f"xh{b}_{k}")
                nc.sync.dma_start(out=t, in_=ago[k])
                xh.append(t)
            return xh

        def reduce_scatter(b, rsin):
            """ReduceScatter the staged rank-major partials; add into residual."""
            rsout = dram.tile([KT, 128, TB], dt.bfloat16, name="rsout", tag=f"rsout{b}")
            if collective_mode == "on":
                nc.gpsimd.collective_compute(
                    "ReduceScatter", mybir.AluOpType.add, replica_groups=RG,
                    ins=[rsin.opt()], outs=[rsout.opt()])
            else:
                nc.gpsimd.dma_start(out=rsout.opt(), in_=rsin[0])
            cs = slice(b * TB, (b + 1) * TB)
            for k in range(KT):
                up = upool.tile([128, TB], dt.bfloat16, name="upd", tag="upd")
                nc.sync.dma_start(out=up, in_=rsout[k])
                nc.vector.tensor_add(out=xt[k][:, cs], in0=xt[k][:, cs], in1=up)

        def attention(b, xh, W):
            """qkv + causal attention + proj partials; returns staged rs input."""
            wqk, wv, wpr, bias_t = W['wqk'], W['wv'], W['wpr'], W['bias']
            qt, kt_t = {}, {}
            for h in range(HL):
                qt[h] = apool.tile([64, N], dt.bfloat16, name=f"qT{h}", tag=f"qT{b}_{h}")
                kt_t[h] = apool.tile([64, N], dt.bfloat16, name=f"kT{h}", tag=f"kT{b}_{h}")
            v_st = {}
            for g in range(2):          # 0 = q, 1 = k
                for cc in range(NCHUNK):
                    ts = slice(cc * CHUNK, (cc + 1) * CHUNK)
                    ps = ps_mm.tile([128, CHUNK], dt.float32, tag="mm")
                    for k in range(KT):
                        nc.tensor.matmul(ps, wqk[k][:, g*128:(g+1)*128], xh[k][:, ts],
                                         start=(k == 0), stop=(k == KT - 1))
                    for h in range(HL):
                        dst = (qt if g == 0 else kt_t)[h]
                        if bias_on["bqk"]:
                            nc.scalar.activation(
                                out=dst[:, ts], in_=ps[h*64:(h+1)*64, :],
                                func=AF.Identity,
                                bias=bias_t["bqk"][h*64:(h+1)*64, g:g+1])
                        else:
                            nc.scalar.activation(out=dst[:, ts],
                                                 in_=ps[h*64:(h+1)*64, :], func=AF.Copy)
            for kti in range(N // 128):   # v, token-major with ones column
                ps = ps_mm.tile([128, CL], dt.float32, tag="mm")
                for k in range(KT):
                    nc.tensor.matmul(ps, xh[k][:, kti*128:(kti+1)*128], wv[k],
                                     start=(k == 0), stop=(k == KT - 1))
                vt = apool.tile([128, HL * VS], dt.bfloat16,
                                name=f"v{kti}", tag=f"v{b}_{kti}")
                vv = vt.rearrange("p (h e) -> p h e", e=VS)
                nc.scalar.activation(out=vv[:, :, 0:D],
                                     in_=ps.rearrange("p (h e) -> p h e", e=D),
                                     func=AF.Copy)
                nc.vector.memset(vv[:, :, D:D+1], 1.0)
                if bias_on["bv"]:
                    nc.vector.tensor_add(
                        out=vv[:, :, 0:D], in0=vv[:, :, 0:D],
                        in1=bias_t["bv"].rearrange("p (h e) -> p h e", e=D))
                v_st[kti] = vt

            ot_sb = {}
            for qc in range(NCHUNK):
                ot_sb[qc] = apool.tile([128, CHUNK], dt.bfloat16,
                                       name=f"oT{qc}", tag=f"oT{b}_{qc}")
            scale = float(D) ** -0.5
            for h in range(HL):
                for qc in range(NCHUNK):
                    qs = slice(qc * CHUNK, (qc + 1) * CHUNK)
                    nkt = (qc + 1) * (CHUNK // 128)
                    ot_ps = ps_ot.tile([65, CHUNK], dt.float32, tag="ot")
                    for kt in range(nkt):
                        s_ps = ps_mm.tile([128, CHUNK], dt.float32, tag="mm")
                        nc.tensor.matmul(s_ps, kt_t[h][:, kt*128:(kt+1)*128],
                                         qt[h][:, qs], start=True, stop=True)
                        es = espool.tile([128, CHUNK], dt.bfloat16, name="es", tag="es")
                        nc.scalar.activation(out=es, in_=s_ps, func=AF.Exp, scale=scale)
                        rel = kt * 128 - qc * CHUNK
                        if rel >= 0:
                            nc.vector.tensor_mul(out=es, in0=es, in1=mask_t[rel // 128])
                        nc.tensor.matmul(ot_ps, v_st[kt][:, h*VS:h*VS+D+1], es,
                                         start=(kt == 0), stop=(kt == nkt - 1))
                    recip = spool.tile([1, CHUNK], dt.float32, tag="recip", bufs=2)
                    nc.vector.reciprocal(out=recip, in_=ot_ps[64:65, :])
                    recb = spool.tile([1, CHUNK], dt.bfloat16, tag="recb", bufs=2)
                    nc.scalar.activation(out=recb, in_=recip, func=AF.Copy)
                    rb_ps = ps_bc.tile([128, CHUNK], dt.float32, tag="bc")
                    nc.tensor.matmul(rb_ps[0:64, :], ones_row[:, 0:64], recb,
                                     start=True, stop=True)
                    rb_sb = spool.tile([64, CHUNK], dt.bfloat16, tag="rb_sb", bufs=2)
                    nc.scalar.activation(out=rb_sb, in_=rb_ps[0:64, :], func=AF.Copy)
                    nc.vector.tensor_mul(out=ot_sb[qc][h*64:(h+1)*64, :],
                                         in0=ot_ps[0:64, :], in1=rb_sb)

            rsin = dram.tile([NCORES, KT, 128, TB], dt.bfloat16, name="rsin",
                             tag=f"rsin{b}")
            for qc in range(NCHUNK):
                for mt in range(KT):
                    ps = ps_mm.tile([128, CHUNK], dt.float32, tag="mm")
                    nc.tensor.matmul(ps, wpr[:, mt*128:(mt+1)*128], ot_sb[qc],
                                     start=True, stop=True)
                    st = stpool.tile([128, CHUNK], dt.bfloat16, name="prst", tag="stage")
                    if bias_on["bpr"]:
                        nc.scalar.activation(out=st, in_=ps, func=AF.Identity,
                                             bias=bias_t["bpr"][:, mt:mt+1])
                    else:
                        nc.scalar.activation(out=st, in_=ps, func=AF.Copy)
                    nr = CHUNK // TB
                    nc.sync.dma_start(
                        out=rsin.rearrange("r k p c -> k p r c")[mt][:, qc*nr:(qc+1)*nr],
                        in_=st)
            return rsin

        def ffn(b, xh, W):
            wf1, wf2, bias_t = W['wf1'], W['wf2'], W['bias']
            ga = {}
            for cc in range(NCHUNK):
                ts = slice(cc * CHUNK, (cc + 1) * CHUNK)
                for mt in range(HIDL // 128):
                    ps = ps_mm.tile([128, CHUNK], dt.float32, tag="mm")
                    for k in range(KT):
                        nc.tensor.matmul(ps, wf1[k][:, mt*128:(mt+1)*128], xh[k][:, ts],
                                         start=(k == 0), stop=(k == KT - 1))
                    at = gpool.tile([128, CHUNK], dt.bfloat16,
                                    name=f"ga{mt}", tag=f"ga{b}_{mt}_{cc}")
                    if gelu_mode == "exact":
                        if bias_on["bf1"]:
                            nc.scalar.activation(out=at, in_=ps, func=AF.Gelu,
                                                 bias=bias_t["bf1"][:, mt:mt+1])
                        else:
                            nc.scalar.activation(out=at, in_=ps, func=AF.Gelu)
                    else:
                        assert not bias_on["bf1"]
                        sg = sqpool.tile([128, CHUNK], dt.bfloat16, name="sg", tag="sq")
                        nc.scalar.activation(out=sg, in_=ps, func=AF.Sigmoid,
                                             scale=1.702)
                        nc.vector.tensor_mul(out=at, in0=sg, in1=ps)
                    ga[(mt, cc)] = at
            rsin = dram.tile([NCORES, KT, 128, TB], dt.bfloat16, name="rsin2",
                             tag=f"rsin{b}")
            for cc in range(NCHUNK):
                for mt in range(KT):
                    ps = ps_mm.tile([128, CHUNK], dt.float32, tag="mm")
                    for j in range(HIDL // 128):
                        nc.tensor.matmul(ps, wf2[j][:, mt*128:(mt+1)*128], ga[(j, cc)],
                                         start=(j == 0), stop=(j == HIDL // 128 - 1))
                    st = stpool.tile([128, CHUNK], dt.bfloat16, name="f2st", tag="stage")
                    if bias_on["bf2"]:
                        nc.scalar.activation(out=st, in_=ps, func=AF.Identity,
                                             bias=bias_t["bf2"][:, mt:mt+1])
                    else:
                        nc.scalar.activation(out=st, in_=ps, func=AF.Copy)
                    nr = CHUNK // TB
                    nc.sync.dma_start(
                        out=rsin.rearrange("r k p c -> k p r c")[mt][:, cc*nr:(cc+1)*nr],
                        in_=st)
            return rsin

        for li in range(L * loop_mult):
            l = li % L
            W = load_weights(l)
            xh = {}
            for b in range(B):
                xb = layer_norm(b, f"ln1_{l}")
                xh[b] = all_gather(b, xb)
            rs1 = {}
            for b in range(B):
                rs1[b] = attention(b, xh[b], W)
                reduce_scatter(b, rs1[b])
            xh2 = {}
            for b in range(B):
                xb2 = layer_norm(b, f"ln2_{l}")
                xh2[b] = all_gather(b, xb2)
            for b in range(B):
                rs2 = ffn(b, xh2[b], W)
                reduce_scatter(b, rs2)

        for k in range(KT):
            o16 = consts.tile([128, 2 * TB], dt.float16, name=f"o16_{k}")
            nc.vector.tensor_copy(out=o16, in_=xt[k])
            nc.sync.dma_start(out=out_p[k], in_=o16)

    if not nc.is_finalized():
        nc.finalize()
    return nc


def _prep_core_inputs(inputs, folded):
    """Per-core in_maps: TP=8 shard + int8 quantization + fp16 token-shard x."""
    x = np.asarray(inputs['x'], np.float32)

    per_core = []
    bias_on = {k: False for k in ("bqk", "bv", "bpr", "bf1", "bf2")}
    for r in range(NCORES):
        wqk_l, wv_l, wpr_l, wf1_l, wf2_l, sc_l = [], [], [], [], [], []
        bqk_l, bv_l, bpr_l, bf1_l, bf2_l = [], [], [], [], []
        for l in range(L):
            F = folded[l]
            sc = np.zeros((128, SC_COLS), np.float32)
            Wq = F['Wqkv_t'][:, r*CL:(r+1)*CL]
            Wk = F['Wqkv_t'][:, C + r*CL: C + (r+1)*CL]
            Wv = F['Wqkv_t'][:, 2*C + r*CL: 2*C + (r+1)*CL]
            q8, s = _q8rows(np.concatenate([Wq, Wk], axis=1))
            wqk_l.append(q8.reshape(KT, 128, 2*CL))
            sc[:, SCQK:SCQK+KT] = s.reshape(KT, 128).T
            q8, s = _q8rows(Wv)
            wv_l.append(q8.reshape(KT, 128, CL))
            sc[:, SCV:SCV+KT] = s.reshape(KT, 128).T
            q8, s = _q8rows(F['Wproj_t'][r*CL:(r+1)*CL, :])
            wpr_l.append(q8)
            sc[:, SCPR] = s
            q8, s = _q8rows(F['Wfc1_t'][:, r*HIDL:(r+1)*HIDL])
            wf1_l.append(q8.reshape(KT, 128, HIDL))
            sc[:, SCF1:SCF1+KT] = s.reshape(KT, 128).T
            q8, s = _q8rows(F['Wfc2_t'][r*HIDL:(r+1)*HIDL, :])
            wf2_l.append(q8.reshape(HIDL // 128, 128, C))
            sc[:, SCF2:SCF2 + HIDL // 128] = s.reshape(HIDL // 128, 128).T
            sc_l.append(sc)
            bq = F['bqkv'][r*CL:(r+1)*CL]
            bk = F['bqkv'][C + r*CL: C + (r+1)*CL]
            bqk_l.append(np.stack([bq, bk], axis=1))                      # [128,2]
            bv_l.append(np.broadcast_to(
                F['bqkv'][2*C + r*CL: 2*C + (r+1)*CL], (128, CL)).copy())
            bpr_l.append(F['bproj'].reshape(KT, 128).T / NCORES)
            bf1_l.append(F['bfc1'][r*HIDL:(r+1)*HIDL].reshape(HIDL // 128, 128).T)
            bf2_l.append(F['bfc2'].reshape(KT, 128).T / NCORES)
        # token-shard of x: tokens [TB*r, TB*(r+1)) of each batch, feature-major
        xs = slice(TB * r, TB * (r + 1))
        xcols = np.concatenate([np.ascontiguousarray(x[b, xs, :].T)[:, :, None]
                                .reshape(C, TB) for b in range(B)], axis=1)
        m = dict(
            x16=xcols.reshape(KT, 128, B * TB).astype(F16),
            wqk=np.stack(wqk_l), wv=np.stack(wv_l), wpr=np.stack(wpr_l),
            wf1=np.stack(wf1_l), wf2=np.stack(wf2_l),
            sc=np.stack(sc_l),
            bqk=np.stack(bqk_l).astype(np.float32),
            bv=np.stack(bv_l).astype(np.float32),
            bpr=np.stack(bpr_l).astype(np.float32),
            bf1=np.stack(bf1_l).astype(np.float32),
            bf2=np.stack(bf2_l).astype(np.float32))
        per_core.append(m)

    for nm in bias_on:
        bias_on[nm] = any(bool(np.abs(m[nm]).max() > 0) for m in per_core)
    for m in per_core:
        for nm in list(m):
            if nm in bias_on and not bias_on[nm]:
                del m[nm]
    return per_core, bias_on


LAST_RESULT = None


def kernel(**inputs):
    global LAST_RESULT
    from concourse.bass_utils import run_bass_kernel_spmd
    folded = _fold_weights(inputs)
    in_maps, bias_on = _prep_core_inputs(inputs, folded)
    nc = build_program(bias_on)
    res = run_bass_kernel_spmd(nc, in_maps, core_ids=list(range(NCORES)))
    LAST_RESULT = res
    out = np.zeros((B, N, C), np.float32)
    for r in range(NCORES):
        o = res.results[r]["out"].reshape(C, B * TB).astype(np.float32)
        for b in range(B):
            out[b, TB*r:TB*(r+1), :] = o[:, b*TB:(b+1)*TB].T
    return out


if __name__ == "__main__":
    import reference
    inp = reference.setup_inputs()
    out = kernel(**{k: np.asarray(v) for k, v in inp.items()})
    exp = np.asarray(reference.reference(**inp))
    err = np.abs(out - exp).max() / np.abs(exp).max()
    print("Relative error:", err)
